# revision 34
# baseline (speedup 1.0000x reference)
"""Trainium2 Bass kernel: single-head attention with RoPE and the reference's
multiplicative causal mask (masked logits stay 0 -> exp(0)=1, so masked
positions contribute exp(0)=1 to softmax -- attention is dense over the
upper triangle too, but those probabilities are a constant 1/Z).

Sharding: 8 cores = 4 batches x 2 row-parity halves. Core (b, h) owns the
interleaved rows x[b, h::2] -- with this split the causal-mask tile classes
are identical on every core, so fully-masked S^T tiles are skipped
statically (same SPMD graph everywhere) and their P==1 contribution enters
as a per-dout constant (onesum) plus a denominator offset.

Per core: project K (dlow-outer, weight panels loaded once), AllGather
roped K within the 2-core pair (single collective on the sync queue,
hidden under the V projection), project V (wb-outer so Wv column blocks
stream in late), AllGather V (hidden under the Q projection), project Q
with cos/sin reused from SBUF, then S^T = K@Q^T, P = exp(mask*S^T/sqrt(S)),
O^T = V^T@P^T / denom. Output is bf16 O^T per core; the host upcasts,
transposes and reassembles.
"""

import sys

for _p in ("/opt/trn_rl_repo", "/root/.axon_site/_ro/trn_rl_repo"):
    if _p not in sys.path:
        sys.path.append(_p)

import math

import ml_dtypes
import numpy as np

BF16 = ml_dtypes.bfloat16

B, S, D = 4, 2048, 2048
NOWN = 1024  # query rows per core
P = 128  # partitions
KD = D // P  # 16 feature chunks
NCJ = S // P  # 16 key chunks
N_CORES = 8
PAIRS = [[0, 1], [2, 3], [4, 5], [6, 7]]
FB = 512  # matmul moving free-dim block
NB = NOWN // FB  # 2 blocks of own rows
SCALE = 1.0 / math.sqrt(S)  # reference scales by sqrt(seq_len), not sqrt(D)

# Quarter-granularity mask staircase (identical on every core with
# interleaved rows): for i-quarter q (256 columns) and j-chunk jc with
# m = (jc % 8) // 2:  q < m -> fully masked (skipped, P==1);
# q == m -> mixed (mask applied); q > m -> fully unmasked.
QW = 256
NQ = NOWN // QW  # 4 quarters


def _m_of(jc):
    return (jc % 8) // 2


# chunks contributing computed S tiles for quarter q
def _comp(q):
    return [jc for jc in range(NCJ) if _m_of(jc) <= q]


# onesum stages: chunks that become skipped when stepping down a quarter
OS_STAGES = [
    [jc for jc in range(NCJ) if _m_of(jc) == 3],  # skipped for q<=2
    [jc for jc in range(NCJ) if _m_of(jc) == 2],  # additionally for q<=1
    [jc for jc in range(NCJ) if _m_of(jc) == 1],  # additionally for q==0
]

_CACHE = {}


def _build():
    import concourse.bass as bass  # noqa: F401
    import concourse.tile as tile
    from concourse import bacc, mybir

    f32 = mybir.dt.float32
    bf16 = mybir.dt.bfloat16

    nc = bacc.Bacc(
        "TRN2", target_bir_lowering=False, debug=False, num_devices=N_CORES
    )

    x_ext = nc.dram_tensor("x_t", [P, KD, NOWN], bf16, kind="ExternalInput").ap()
    wq_ext = nc.dram_tensor("wq", [KD, P, KD, P], bf16, kind="ExternalInput").ap()
    wk_ext = nc.dram_tensor("wk", [KD, P, KD, P], bf16, kind="ExternalInput").ap()
    wv_ext = nc.dram_tensor("wv", [P, KD, D], bf16, kind="ExternalInput").ap()
    cos_ext = nc.dram_tensor("cos_t", [KD, P, NOWN], bf16, kind="ExternalInput").ap()
    sin_ext = nc.dram_tensor("sin_t", [KD, P, NOWN], bf16, kind="ExternalInput").ap()
    mask_ext = nc.dram_tensor("mask_t", [NCJ, P, QW], bf16, kind="ExternalInput").ap()
    out_ext = nc.dram_tensor("out", [D, NOWN], bf16, kind="ExternalOutput").ap()

    with tile.TileContext(nc) as tc:
        with (
            tc.tile_pool(name="dram", bufs=1, space="DRAM") as dram,
            tc.tile_pool(name="psum", bufs=5, space="PSUM") as psum,
            tc.tile_pool(name="dnsum", bufs=1, space="PSUM") as dnsum,
            tc.tile_pool(name="persist", bufs=1) as persist,
            tc.tile_pool(name="tmp", bufs=8) as tmp,
            tc.tile_pool(name="strm", bufs=6) as strm,
        ):
            kt_local = dram.tile([NCJ // 2, P, KD, P], bf16)
            v_local = dram.tile([NCJ // 2, P, D], bf16)
            # gathered tensors split in chunk-halves: two 2MB AllGathers
            # run at ~2x the per-op bandwidth of one 4MB op
            kt_ga = dram.tile([2, 4, P, KD, P], bf16)
            kt_gb = dram.tile([2, 4, P, KD, P], bf16)
            v_ga = dram.tile([2, 4, P, D], bf16)
            v_gb = dram.tile([2, 4, P, D], bf16)

            def kt_g(jc):
                h2, jcl = jc // 8, jc % 8
                return (kt_ga if jcl < 4 else kt_gb)[h2, jcl % 4]

            def v_g(jc):
                h2, jcl = jc // 8, jc % 8
                return (v_ga if jcl < 4 else v_gb)[h2, jcl % 4]

            ones_col = persist.tile([P, 1], bf16)
            nc.vector.memset(ones_col, 1.0)
            ones_row = persist.tile([1, P], f32)
            nc.vector.memset(ones_row, 1.0)

            # dummy full-array matmuls during the input-DMA head: the PE
            # activity monitor un-throttles after ~3.4us of sustained
            # work, so the first real chains run at 2.4GHz instead of 1.2
            warm = persist.tile([P, FB], bf16)
            nc.vector.memset(warm, 0.0)
            ps_w = psum.tile([P, FB], f32, tag="ps", name="warm")
            for _ in range(12):
                nc.tensor.matmul(
                    ps_w, lhsT=warm[:, 0:P], rhs=warm, start=True, stop=True
                )

            # round-robin engine cycling for small DMA triggers, so no
            # single engine queue saturates on dispatch cost
            _rr = [0]
            _rr_engines = [nc.gpsimd, nc.sync, nc.scalar]

            def rr_dma(out, in_):
                _rr_engines[_rr[0] % 3].dma_start(out=out, in_=in_)
                _rr[0] += 1

            # x in 16 half-tiles [P, 2, FB] so the first projection chain
            # only waits on the first column-half (2MB) of x
            x_pool = tc.alloc_tile_pool(name="x_pool", bufs=1)
            x_ts = [
                [
                    x_pool.tile([P, 2, FB], bf16, name=f"x_sb{kg}_{h}")
                    for h in range(NB)
                ]
                for kg in range(8)
            ]
            _x_engines = [nc.gpsimd, nc.sync, nc.scalar]

            def emit_x_half(h):
                # per-chunk DMAs in chain order: MM k of the first chain
                # only waits for its own 128KB chunk, so the PE starts
                # early and streams behind the DMA front
                for k in range(KD):
                    _x_engines[k % len(_x_engines)].dma_start(
                        out=x_ts[k // 2][h][:, k % 2 : k % 2 + 1, :],
                        in_=x_ext[:, k : k + 1, h * FB : (h + 1) * FB],
                    )

            def x_nb(k, nbi):
                # [P, FB] block of x chunk k, columns nbi*FB..
                return x_ts[k // 2][nbi][:, k % 2, :]

            def x_col(k, ncc):
                # [P, P] chunk-column ncc of x chunk k
                h, r = ncc // 4, ncc % 4
                return x_ts[k // 2][h][:, k % 2, r * P : (r + 1) * P]

            def load_panels(wpool, w_ext, dlow, e0, e1):
                dhigh = dlow + KD // 2
                w_lo = wpool.tile([P, KD, P], bf16, tag="wp", name=f"wlo{dlow}")
                e0.dma_start(out=w_lo, in_=w_ext[dlow])
                w_hi = wpool.tile([P, KD, P], bf16, tag="wp", name=f"whi{dlow}")
                e1.dma_start(out=w_hi, in_=w_ext[dhigh])
                return w_lo, w_hi

            # cos/sin loaded once (full row range) per dlow and kept in
            # SBUF for both the K and Q projections
            cs_pool = tc.alloc_tile_pool(name="cs_pool", bufs=1)
            cs_all = {}

            def cs_load_full(dlow):
                dhigh = dlow + KD // 2
                tiles = []
                for i, (name, src_) in enumerate(
                    (
                        ("ct", cos_ext[dlow]),
                        ("st", sin_ext[dlow]),
                        ("ch", cos_ext[dhigh]),
                        ("sh", sin_ext[dhigh]),
                    )
                ):
                    t = cs_pool.tile([P, NOWN], bf16, name=f"cs_{name}{dlow}")
                    (nc.sync if i % 2 == 0 else nc.scalar).dma_start(
                        out=t, in_=src_
                    )
                    tiles.append(t)
                cs_all[dlow] = tiles
                return tiles

            def rope_pair(panels, dlow, nb, cs_tiles, out_ap, post):
                """One (dlow, nb) unit: two projection chains + rope."""
                dhigh = dlow + KD // 2
                sl = slice(nb * FB, (nb + 1) * FB)
                cos_t, sin_t, cos_h, sin_h = (t[:, sl] for t in cs_tiles)
                w_lo, w_hi = panels
                ps_lo = psum.tile([P, FB], f32, tag="ps", name=f"plo{dlow}{nb}")
                for k in range(KD):
                    nc.tensor.matmul(
                        ps_lo,
                        lhsT=w_lo[:, k, :],
                        rhs=x_nb(k, nb),
                        start=(k == 0),
                        stop=(k == KD - 1),
                    )
                ps_hi = psum.tile([P, FB], f32, tag="ps", name=f"phi{dlow}{nb}")
                for k in range(KD):
                    nc.tensor.matmul(
                        ps_hi,
                        lhsT=w_hi[:, k, :],
                        rhs=x_nb(k, nb),
                        start=(k == 0),
                        stop=(k == KD - 1),
                    )
                # rope low half: out = lo*cos_l - hi*sin_l
                t1 = tmp.tile([P, FB], f32, tag="t", name=f"t1{dlow}{nb}")
                nc.vector.tensor_mul(t1, ps_lo, cos_t)
                t2 = tmp.tile([P, FB], f32, tag="t", name=f"t2{dlow}{nb}")
                nc.vector.tensor_mul(t2, ps_hi, sin_t)
                o_lo = out_ap(dlow, nb)
                nc.vector.tensor_sub(o_lo, t1, t2)
                if post is not None:
                    post(dlow, nb, o_lo)
                # rope high half: out = hi*cos_h + lo*sin_h
                t3 = tmp.tile([P, FB], f32, tag="t", name=f"t3{dlow}{nb}")
                nc.vector.tensor_mul(t3, ps_hi, cos_h)
                t4 = tmp.tile([P, FB], f32, tag="t", name=f"t4{dlow}{nb}")
                nc.vector.tensor_mul(t4, ps_lo, sin_h)
                o_hi = out_ap(dhigh, nb)
                nc.vector.tensor_add(o_hi, t3, t4)
                if post is not None:
                    post(dhigh, nb, o_hi)

            # ---- K projection + rope -> kt_local ----
            def k_out(dc, nb):
                return strm.tile([P, FB], bf16, tag="ro", name=f"kt_{dc}_{nb}")

            def k_post(dc, nb, t):
                for jj in range(FB // P):
                    rr_dma(
                        kt_local[nb * 4 + jj][:, dc, :],
                        t[:, jj * P : (jj + 1) * P],
                    )

            # wv column blocks [P, KD, FB]; blocks 0/1 start loading during
            # the last K units (own pool so only 32KB/part overlaps the wk
            # panel pool), blocks 2/3 at V start after wk_pool closes
            wv_blks = {}

            def emit_wv_load(pool, wb, eng):
                t = pool.tile([P, KD, FB], bf16, name=f"wv_sb{wb}")
                eng.dma_start(out=t, in_=wv_ext[:, :, wb * FB : (wb + 1) * FB])
                wv_blks[wb] = t

            # dlow-outer with one-unit prefetch of panels + cos/sin
            _pe = [nc.sync, nc.scalar]
            wv_pool_a = tc.alloc_tile_pool(name="wv_pool_a", bufs=1)
            with tc.tile_pool(name="wk_pool", bufs=6) as wkp:
                # priority order for the startup DMA front: first-unit
                # panels, then x half0 (first chains), then cos/sin (only
                # needed ~3.4us in), then x half1
                panels_next = load_panels(wkp, wk_ext, 0, nc.sync, nc.scalar)
                emit_x_half(0)
                cs_next = cs_load_full(0)
                emit_x_half(1)
                for dlow in range(KD // 2):
                    panels, cs_full = panels_next, cs_next
                    if dlow + 1 < KD // 2:
                        e0 = _pe[(dlow + 1) % 2]
                        e1 = _pe[dlow % 2]
                        panels_next = load_panels(wkp, wk_ext, dlow + 1, e0, e1)
                        cs_next = cs_load_full(dlow + 1)
                    if dlow == 6:
                        emit_wv_load(wv_pool_a, 0, nc.gpsimd)
                    if dlow == 7:
                        emit_wv_load(wv_pool_a, 1, nc.gpsimd)
                    for nb in range(NB):
                        rope_pair(panels, dlow, nb, cs_full, k_out, k_post)

            # pair-AllGathers of roped K^T; the doorbell writes on gpsimd
            # wait only for the kt_local writes, and nothing CC-dependent
            # sits on gpsimd until the v2 loads much later
            for half, out_t in ((0, kt_ga), (1, kt_gb)):
                nc.gpsimd.collective_compute(
                    "AllGather",
                    mybir.AluOpType.bypass,
                    replica_groups=PAIRS,
                    ins=[kt_local[half * 4 : (half + 1) * 4].opt()],
                    outs=[out_t.opt()],
                )

            # ---- V projection (wb-outer; Wv blocks stream in) ----
            wv_pool_b = tc.alloc_tile_pool(name="wv_pool_b", bufs=1)
            emit_wv_load(wv_pool_b, 2, nc.scalar)
            emit_wv_load(wv_pool_b, 3, nc.gpsimd)
            for wb in range(D // FB):
                for ncc in range(NCJ // 2):
                    ps_v = psum.tile([P, FB], f32, tag="ps")
                    for k in range(KD):
                        nc.tensor.matmul(
                            ps_v,
                            lhsT=x_col(k, ncc),
                            rhs=wv_blks[wb][:, k, :],
                            start=(k == 0),
                            stop=(k == KD - 1),
                        )
                    v_t = strm.tile([P, FB], bf16, tag="vo")
                    nc.vector.tensor_copy(v_t, ps_v)
                    rr_dma(v_local[ncc][:, wb * FB : (wb + 1) * FB], v_t)
            wv_pool_b.release()
            wv_pool_a.release()

            for half, out_t in ((0, v_ga), (1, v_gb)):
                nc.gpsimd.collective_compute(
                    "AllGather",
                    mybir.AluOpType.bypass,
                    replica_groups=PAIRS,
                    ins=[v_local[half * 4 : (half + 1) * 4].opt()],
                    outs=[out_t.opt()],
                )

            # ---- Q projection + rope (cos/sin reused from SBUF) ----
            # qt + slab pools live on the right SBUF stack, created before
            # Q so they sit OUTSIDE the zone freed by x/cs: the kt slab
            # loads can then prefetch during the Q projection instead of
            # picking up a runtime dependency on the x/cs release
            qt_pool = tc.alloc_tile_pool(name="qt_pool", bufs=1, side="right")
            qt_sb = qt_pool.tile([P, KD, NOWN], bf16)
            slab = tc.alloc_tile_pool(name="slab", bufs=4, side="right")

            def q_out(dc, nb):
                return qt_sb[:, dc, nb * FB : (nb + 1) * FB]

            # wq panels on scalar only: gpsimd holds the v2 loads that wait
            # on the V-gather, sync holds the kt-slab loads that wait on
            # the K-gather -- panels must not queue behind either
            with tc.tile_pool(name="wq_pool", bufs=6) as wqp:
                q_panels_next = load_panels(wqp, wq_ext, 0, nc.scalar, nc.scalar)
                for dlow in range(KD // 2):
                    panels = q_panels_next
                    if dlow + 1 < KD // 2:
                        q_panels_next = load_panels(
                            wqp, wq_ext, dlow + 1, nc.scalar, nc.scalar
                        )
                    for nb in range(NB):
                        rope_pair(panels, dlow, nb, cs_all[dlow], q_out, None)
            cs_pool.release()
            x_pool.release()

            # ---- Attention ----
            with (
                tc.tile_pool(name="v2_pool", bufs=1, side="right") as v2p,
                tc.tile_pool(name="pt_pool", bufs=1, side="right") as ptp,
                tc.tile_pool(name="mskp", bufs=3, side="right") as mskp,
                tc.tile_pool(name="outp", bufs=4, side="right") as outp,
                tc.tile_pool(name="smallp", bufs=2, side="right") as smallp,
            ):
                v2_sb = v2p.tile([P, NCJ, D], bf16)
                for jc in range(NCJ):
                    nc.gpsimd.dma_start(out=v2_sb[:, jc, :], in_=v_g(jc))

                pt_sb = ptp.tile([P, NCJ, NOWN], bf16)

                def s_tile(jc, q, kt_slab, msk):
                    sl = slice(q * QW, (q + 1) * QW)
                    ps_s = psum.tile([P, QW], f32, tag="ps", name=f"ps_s{jc}{q}")
                    for k in range(KD):
                        nc.tensor.matmul(
                            ps_s,
                            lhsT=kt_slab[:, k, :],
                            rhs=qt_sb[:, k, sl],
                            start=(k == 0),
                            stop=(k == KD - 1),
                        )
                    if msk is not None:
                        tm = tmp.tile([P, QW], f32, tag="t", name=f"tm{jc}{q}")
                        nc.vector.tensor_mul(tm, ps_s, msk)
                        esrc = tm
                    else:
                        esrc = ps_s
                    nc.scalar.activation(
                        out=pt_sb[:, jc, sl],
                        in_=esrc,
                        func=mybir.ActivationFunctionType.Exp,
                        scale=SCALE,
                    )

                # denominators + reciprocals; skipped chunks contribute
                # (12 - 4q)*128 exact ones
                recips = [None] * NQ
                rbs = {}

                def emit_denom(q):
                    jcs = _comp(q)
                    ps_d = dnsum.tile([1, QW], f32, tag="dn", name=f"psd{q}")
                    for idx, jc in enumerate(jcs):
                        nc.tensor.matmul(
                            ps_d,
                            lhsT=ones_col,
                            rhs=pt_sb[:, jc, q * QW : (q + 1) * QW],
                            start=(idx == 0),
                            stop=(idx == len(jcs) - 1),
                        )
                    recip = smallp.tile([1, QW], f32, tag=f"rc{q}", name=f"rc{q}")
                    nones = (12 - 4 * q) * P
                    if nones:
                        dfix = smallp.tile([1, QW], f32, tag="dfix", name=f"df{q}")
                        nc.vector.tensor_scalar_add(dfix, ps_d, float(nones))
                        nc.vector.reciprocal(recip, dfix)
                    else:
                        nc.vector.reciprocal(recip, ps_d)
                    recips[q] = recip

                def emit_rb(q):
                    # reciprocal broadcast via fp32 outer product; emitted
                    # one group after its denom so the DVE recip is done
                    ps_rb = dnsum.tile([P, QW], f32, tag="rb", name=f"prb{q}")
                    nc.tensor.matmul(
                        ps_rb, lhsT=ones_row, rhs=recips[q], start=True, stop=True
                    )
                    # distinct tag per q: all four broadcasts stay live
                    # until their q's scale pass at the end of the kernel
                    rb = smallp.tile([P, QW], f32, tag=f"rbs{q}", name=f"rb{q}")
                    nc.vector.tensor_copy(rb, ps_rb)
                    rbs[q] = rb

                # onesum partials in one PSUM tile [P, 3*KD], emitted as
                # (stage, dc) groups of 4 tiny MMs interleaved among the
                # later S^T chains (v2 is loaded by then): a contiguous
                # 12us run of tiny MMs looks idle to the PE activity
                # monitor and re-throttles the clock to 1.2GHz
                ps_os3 = dnsum.tile([P, 3 * KD], f32, tag="os")
                os_groups = [(si, dc) for si in range(3) for dc in range(KD)]
                _gi = [0]

                def emit_os_groups(n):
                    while n > 0 and _gi[0] < len(os_groups):
                        si, dc = os_groups[_gi[0]]
                        _gi[0] += 1
                        n -= 1
                        stage = OS_STAGES[si]
                        col = si * KD + dc
                        for idx, jc in enumerate(stage):
                            nc.tensor.matmul(
                                ps_os3[:, col : col + 1],
                                lhsT=v2_sb[:, jc, dc * P : (dc + 1) * P],
                                rhs=ones_col,
                                start=(idx == 0),
                                stop=(idx == len(stage) - 1),
                            )

                # S^T in mask-group order: after group g, the quarter-g
                # denominator inputs are complete, so the denom/recip/rb
                # work interleaves into the dense S^T stream
                for g in range(NQ):
                    for jc in range(NCJ):
                        if _m_of(jc) != g:
                            continue
                        kt_slab = slab.tile([P, KD, P], bf16, tag="slab")
                        nc.sync.dma_start(out=kt_slab, in_=kt_g(jc))
                        msk = mskp.tile([P, QW], bf16, tag="m")
                        nc.scalar.dma_start(out=msk, in_=mask_ext[jc])
                        s_tile(jc, g, kt_slab, msk)  # the mixed quarter
                        for q in range(g + 1, NQ):
                            s_tile(jc, q, kt_slab, None)  # fully unmasked
                        if g >= 2:
                            emit_os_groups(6)
                    emit_denom(g)
                    if g > 0:
                        emit_rb(g - 1)
                emit_rb(NQ - 1)
                emit_os_groups(len(os_groups))  # any remainder

                def pv_chain(q, dc):
                    jcs = _comp(q)
                    ps_o = psum.tile([P, QW], f32, tag="ps", name=f"pso{q}{dc}")
                    for idx, jc in enumerate(jcs):
                        nc.tensor.matmul(
                            ps_o,
                            lhsT=v2_sb[:, jc, dc * P : (dc + 1) * P],
                            rhs=pt_sb[:, jc, q * QW : (q + 1) * QW],
                            start=(idx == 0),
                            stop=(idx == len(jcs) - 1),
                        )
                    return ps_o

                os_of_q = {}
                # sync is free once the kt slabs are in, so all three
                # trigger queues share the output drain
                _out_engines = [nc.gpsimd, nc.scalar, nc.sync]

                def emit_scale(q, dc, ps_o):
                    o_st = outp.tile([P, QW], bf16, tag="o", name=f"ost{q}{dc}")
                    if q in os_of_q:
                        nc.vector.scalar_tensor_tensor(
                            out=o_st,
                            in0=ps_o,
                            scalar=os_of_q[q][:, dc : dc + 1],
                            in1=rbs[q],
                            op0=mybir.AluOpType.add,
                            op1=mybir.AluOpType.mult,
                        )
                    else:
                        nc.vector.tensor_mul(o_st, ps_o, rbs[q])
                    _out_engines[dc % 3].dma_start(
                        out=out_ext[
                            dc * P : (dc + 1) * P, q * QW : (q + 1) * QW
                        ],
                        in_=o_st,
                    )

                # first PV chains issue before the DVE onesum combines so
                # the tensor stream stays dense across the transition
                pend = [pv_chain(0, dc) for dc in range(3)]

                # cumulative onesum sums on DVE:
                # q2 uses p0, q1 p0+p1, q0 p0+p1+p2
                parts = []
                for si in range(3):
                    p_sb = smallp.tile([P, KD], f32, tag=f"osp{si}", name=f"osp{si}")
                    nc.vector.tensor_copy(p_sb, ps_os3[:, si * KD : (si + 1) * KD])
                    parts.append(p_sb)
                os1 = smallp.tile([P, KD], f32, tag="os1c")
                nc.vector.tensor_add(os1, parts[0], parts[1])
                os0 = smallp.tile([P, KD], f32, tag="os0c")
                nc.vector.tensor_add(os0, os1, parts[2])
                os_of_q[2] = parts[0]
                os_of_q[1] = os1
                os_of_q[0] = os0

                # PV in ascending q: the longest chains (q=3, 16 MMs) run
                # last, so the output drain overlaps them instead of
                # piling up behind the short q=0 chains at the very end
                for dc in range(3):
                    emit_scale(0, dc, pend[dc])
                for dc in range(3, KD):
                    emit_scale(0, dc, pv_chain(0, dc))
                for q in range(1, NQ):
                    for dc in range(KD):
                        emit_scale(q, dc, pv_chain(q, dc))
            slab.release()
            qt_pool.release()

    nc.compile()
    return nc


def _prep_inputs(x, cos, sin, Wq, Wk, Wv):
    """Host-side sharding/layout prep. Returns in_maps for 8 cores."""
    x = np.asarray(x, dtype=np.float32)
    cos = np.asarray(cos, dtype=np.float32)
    sin = np.asarray(sin, dtype=np.float32)

    def w_panels(w):
        # W.T [din, dout] -> [dc, p_din, k_din, c_dout] with d = k*128+p
        wt = np.ascontiguousarray(np.asarray(w, dtype=np.float32).T).astype(BF16)
        return np.ascontiguousarray(
            wt.reshape(KD, P, KD, P).transpose(2, 1, 0, 3)
        )

    wq_p = w_panels(Wq)
    wk_p = w_panels(Wk)
    # Wv.T [din, dout] -> [p, k, dout]
    wv_p = np.ascontiguousarray(
        np.asarray(Wv, dtype=np.float32)
        .T.astype(BF16)
        .reshape(KD, P, D)
        .transpose(1, 0, 2)
    )

    # global row index of gathered slot s: pair rank h2 = s // NOWN owns the
    # rows with parity h2, so j_global(s) = 2*(s % NOWN) + h2
    slot = np.arange(S, dtype=np.int64)
    j_global = 2 * (slot % NOWN) + slot // NOWN

    in_maps = []
    for c in range(N_CORES):
        b, h = divmod(c, 2)
        rows = slice(h, None, 2)  # interleaved rows: h, h+2, h+4, ...
        xt = np.ascontiguousarray(
            x[b, rows, :].T.astype(BF16).reshape(KD, P, NOWN).transpose(1, 0, 2)
        )
        cos_t = np.ascontiguousarray(cos[rows].T.astype(BF16).reshape(KD, P, NOWN))
        sin_t = np.ascontiguousarray(sin[rows].T.astype(BF16).reshape(KD, P, NOWN))
        i_global = 2 * np.arange(NOWN, dtype=np.int64) + h
        # per jc, only the "mixed" i-quarter needs mask data
        mask_t = np.empty((NCJ, P, QW), dtype=BF16)
        for jc in range(NCJ):
            q = (jc % 8) // 2
            jg = j_global[jc * P : (jc + 1) * P][:, None]
            ig = i_global[q * QW : (q + 1) * QW][None, :]
            mask_t[jc] = (jg <= ig).astype(BF16)
        in_maps.append(
            {
                "x_t": xt,
                "wq": wq_p,
                "wk": wk_p,
                "wv": wv_p,
                "cos_t": cos_t,
                "sin_t": sin_t,
                "mask_t": mask_t,
            }
        )
    return in_maps


def _run(in_maps, trace=False, tmpdir=None):
    from concourse.bass_utils import run_bass_kernel_spmd

    if "nc" not in _CACHE:
        _CACHE["nc"] = _build()
    nc = _CACHE["nc"]
    return run_bass_kernel_spmd(
        nc, in_maps, list(range(N_CORES)), trace=trace, tmpdir=tmpdir
    )


def kernel(x, cos, sin, Wq, Wk, Wv):
    in_maps = _prep_inputs(x, cos, sin, Wq, Wk, Wv)
    res = _run(in_maps, trace=False)
    out = np.empty((B, S, D), dtype=np.float32)
    for c in range(N_CORES):
        b, h = divmod(c, 2)
        out[b, h::2, :] = res.results[c]["out"].astype(np.float32).T
    return out
